# revision 41
# baseline (speedup 1.0000x reference)
"""BatchBlur_SV kernel for 8 Trainium2 NeuronCores (Bass/Tile).

Reference semantics (including its reshape-scrambling "bug"):
  X = ker.reshape(361, 65536)                  # (kernel-pos ab, pixel p)
  s1 = X.sum(0);  W  = X / s1                  # stage-1 per-pixel normalize
  A2 = W.flat chunks of 361; s2 = row sums;  B2 = A2 / s2     # stage 2
  A3 = (B2.T).flat chunks of 361; s3 = row sums               # stage 3
  U  = im2col(reflect_pad(input[0,2], 9)) in (ab, p) layout   # (361, 65536)
  out[r] = sum(U.flat_chunk_r * A3[r]) / s3[r]

All arithmetic runs on-device in 3 SPMD launches over 8 cores. Host only
slices / rolls / transposes / dtype-converts between launches (data
movement, no math).

Big streams travel as fp16 (values are bounded and the gate is rel_err
< 2e-2); accumulations are fp32.  Column-direction reductions (k1 colsum,
k3 dots/s3) run on the PE array as ones-vector matmuls over a transposed
band layout, which is nearly free next to DVE reductions.
"""

import numpy as np

P = 65536          # pixels
L = 19
L2 = 361           # kernel positions
NCORES = 8
PS = P // NCORES   # 8192 rows per core
NB = PS * L2       # flat elements per band
G = 4              # subtiles per DMA group
NGRP = PS // (128 * G)   # 8 groups per core
NC = PS // 512     # 512-wide psum chunks per band (16)
CW = PS // 128     # per-partition width of [128, CW] strip relayout (64)

_CACHE: dict = {}


def _dt():
    from concourse import mybir
    return mybir.dt


def _grouped(ap):
    # (PS, L2) -> [g][k][(i j)] with row = g*1024 + k*G + i: each partition
    # holds G consecutive rows, so src/dst DMA patterns are contiguous 2D.
    return ap.rearrange("(g k i) j -> g k (i j)", g=NGRP, k=128, i=G)


_ROWS = [(0, 128), (128, 256), (256, L2)]  # partition tiles over the 361 rows
_BW = 2048                                 # column block width
_NBLK = PS // _BW                          # 4 blocks per band


def _build_k1():
    """colsum kernel: in xp (L2, PS) = X[:, pband] transposed slab;
    out inv1 (1, PS) strip with inv1[0, i] = 1 / s1[pband_start + i].
    Column sums run on PE as ones-vector matmuls, pipelined in 2048-wide
    column blocks; reciprocals run on the Pool engine per 512-chunk."""
    import concourse.bacc as bacc
    import concourse.tile as tile
    from concourse import mybir

    dt = _dt()
    nc = bacc.Bacc("TRN2", target_bir_lowering=False)
    xp = nc.dram_tensor("xp", [L2, PS], dt.float8e4, kind="ExternalInput")
    inv1 = nc.dram_tensor("inv1", [1, PS], dt.float32, kind="ExternalOutput")
    with tile.TileContext(nc) as tc:
        with (
            tc.tile_pool(name="io", bufs=4) as pool,
            tc.tile_pool(name="st", bufs=1) as spool,
            tc.psum_pool(name="ps", bufs=4) as psp,
        ):
            ones = spool.tile([128, 1], dt.float8e4)
            nc.vector.memset(ones, 1.0)
            strip = spool.tile([1, PS], dt.float16)
            istrip = spool.tile([1, PS], dt.float32)
            for b in range(_NBLK):
                bsl = slice(_BW * b, _BW * (b + 1))
                xcs = []
                for t, (r0, r1) in enumerate(_ROWS):
                    xc = pool.tile([r1 - r0, _BW], dt.float8e4, tag=f"x{t}")
                    nc.sync.dma_start(out=xc, in_=xp[r0:r1, bsl])
                    xcs.append(xc)
                for s in range(_BW // 1024):
                    c = (_BW // 1024) * b + s
                    csl = slice(1024 * c, 1024 * (c + 1))
                    ps = psp.tile([1, 1024], dt.float32)
                    for h in range(2):
                        hsl = slice(512 * h, 512 * (h + 1))
                        xsl = slice(1024 * s + 512 * h, 1024 * s + 512 * (h + 1))
                        for t, (r0, r1) in enumerate(_ROWS):
                            nc.tensor.matmul(
                                ps[:, hsl], lhsT=ones[: r1 - r0, :],
                                rhs=xcs[t][:, xsl],
                                start=(t == 0), stop=(t == 2),
                            )
                    nc.scalar.copy(out=strip[:, csl], in_=ps[:, :])
                nc.vector.reciprocal(out=istrip[:, bsl], in_=strip[:, bsl])
                nc.gpsimd.dma_start(out=inv1[:, bsl], in_=istrip[:, bsl])
    nc.compile()
    return nc


def _build_k2():
    """stage-2 kernel: in a2 (PS,361) = X.flat band, i1b (PS,361) = matching
    per-element stage-1 reciprocal sums; out b2 (PS,361) normalized chunks."""
    import concourse.bacc as bacc
    import concourse.tile as tile
    from concourse import mybir

    dt = _dt()
    nc = bacc.Bacc("TRN2", target_bir_lowering=False)
    a2 = nc.dram_tensor("a2", [PS, L2], dt.float16, kind="ExternalInput")
    i1b = nc.dram_tensor("i1b", [PS, L2], dt.float16, kind="ExternalInput")
    b2 = nc.dram_tensor("b2", [PS, L2], dt.float16, kind="ExternalOutput")
    a2r, i1r, b2r = _grouped(a2[:, :]), _grouped(i1b[:, :]), _grouped(b2[:, :])
    with tile.TileContext(nc) as tc:
        with (
            tc.tile_pool(name="io", bufs=3) as pool,
            tc.tile_pool(name="w", bufs=5) as wpool,
            tc.tile_pool(name="st", bufs=3) as spool,
        ):
            for g in range(NGRP):
                ta = pool.tile([128, G, L2], dt.float16, tag="ta")
                ti = pool.tile([128, G, L2], dt.float16, tag="ti")
                nc.sync.dma_start(
                    out=ta[:, :, :].rearrange("k i j -> k (i j)"), in_=a2r[g]
                )
                nc.scalar.dma_start(
                    out=ti[:, :, :].rearrange("k i j -> k (i j)"), in_=i1r[g]
                )
                tw = wpool.tile([128, G, L2], dt.float16, tag="tw")
                nc.vector.tensor_mul(out=tw, in0=ta, in1=ti)
                s2 = spool.tile([128, G], dt.float32, tag="s2")
                nc.vector.tensor_reduce(
                    out=s2, in_=tw,
                    axis=mybir.AxisListType.X, op=mybir.AluOpType.add,
                )
                r2 = spool.tile([128, G], dt.float32, tag="r2")
                nc.vector.reciprocal(out=r2, in_=s2)
                tb = wpool.tile([128, G, L2], dt.float16, tag="tb")
                for i in range(G):
                    eng = nc.vector.tensor_scalar_mul if i % 2 else None
                    if eng is None:
                        nc.scalar.mul(
                            out=tb[:, i, :], in_=tw[:, i, :],
                            mul=r2[:, i : i + 1],
                        )
                    else:
                        eng(
                            out=tb[:, i, :], in0=tw[:, i, :],
                            scalar1=r2[:, i : i + 1],
                        )
                nc.gpsimd.dma_start(
                    out=b2r[g], in_=tb[:, :, :].rearrange("k i j -> k (i j)")
                )
    nc.compile()
    return nc


def _build_k3():
    """final kernel: in vT/uT (L2, PS) = transposed B2T/U flat bands;
    out o (1, PS) strip with o[0, i] = out[band_start + i]."""
    import concourse.bacc as bacc
    import concourse.tile as tile
    from concourse import mybir

    dt = _dt()
    nc = bacc.Bacc("TRN2", target_bir_lowering=False)
    vT = nc.dram_tensor("vT", [L2, PS], dt.float16, kind="ExternalInput")
    uT = nc.dram_tensor("uT", [L2, PS], dt.float16, kind="ExternalInput")
    o = nc.dram_tensor("o", [128, CW], dt.float32, kind="ExternalOutput")
    scd = nc.dram_tensor("scd", [1, PS], dt.float32, kind="Internal")
    scs = nc.dram_tensor("scs", [1, PS], dt.float16, kind="Internal")
    with tile.TileContext(nc) as tc:
        with (
            tc.tile_pool(name="io", bufs=4) as pool,
            tc.tile_pool(name="pr", bufs=3) as prp,
            tc.tile_pool(name="st", bufs=2) as spool,
            tc.psum_pool(name="ps", bufs=2) as psp,
        ):
            ones = spool.tile([128, 1], dt.float16)
            nc.vector.memset(ones, 1.0)
            for b in range(_NBLK):
                bsl = slice(_BW * b, _BW * (b + 1))
                dstr = spool.tile([1, _BW], dt.float32, tag="dstr")
                sstr = spool.tile([1, _BW], dt.float16, tag="sstr")
                vcs, prods = [], []
                for t, (r0, r1) in enumerate(_ROWS):
                    vc = pool.tile([r1 - r0, _BW], dt.float16, tag=f"v{t}")
                    uc = pool.tile([r1 - r0, _BW], dt.float16, tag=f"u{t}")
                    nc.sync.dma_start(out=vc, in_=vT[r0:r1, bsl])
                    nc.scalar.dma_start(out=uc, in_=uT[r0:r1, bsl])
                    pr = prp.tile([r1 - r0, _BW], dt.float16, tag=f"p{t}")
                    nc.vector.tensor_mul(out=pr, in0=vc, in1=uc)
                    vcs.append(vc)
                    prods.append(pr)
                for s in range(_BW // 1024):
                    sl = slice(1024 * s, 1024 * (s + 1))
                    psd = psp.tile([1, 1024], dt.float32, tag="psd")
                    pss = psp.tile([1, 1024], dt.float32, tag="pss")
                    for h in range(2):
                        hsl = slice(512 * h, 512 * (h + 1))
                        xsl = slice(1024 * s + 512 * h, 1024 * s + 512 * (h + 1))
                        for t, (r0, r1) in enumerate(_ROWS):
                            nc.tensor.matmul(
                                psd[:, hsl], lhsT=ones[: r1 - r0, :],
                                rhs=prods[t][:, xsl],
                                start=(t == 0), stop=(t == 2),
                            )
                        for t, (r0, r1) in enumerate(_ROWS):
                            nc.tensor.matmul(
                                pss[:, hsl], lhsT=ones[: r1 - r0, :],
                                rhs=vcs[t][:, xsl],
                                start=(t == 0), stop=(t == 2),
                            )
                    nc.scalar.copy(out=dstr[:, sl], in_=psd[:, :])
                    nc.scalar.copy(out=sstr[:, sl], in_=pss[:, :])
                nc.gpsimd.dma_start(out=scd[:, bsl], in_=dstr)
                nc.gpsimd.dma_start(out=scs[:, bsl], in_=sstr)
            td = spool.tile([128, CW], dt.float32, tag="td")
            ts = spool.tile([128, CW], dt.float16, tag="ts")
            nc.sync.dma_start(
                out=td, in_=scd[:, :].rearrange("a (k c) -> (a k) c", k=128)
            )
            nc.scalar.dma_start(
                out=ts, in_=scs[:, :].rearrange("a (k c) -> (a k) c", k=128)
            )
            tr = spool.tile([128, CW], dt.float32, tag="tr")
            nc.vector.reciprocal(out=tr, in_=ts)
            to = spool.tile([128, CW], dt.float32, tag="to")
            nc.vector.tensor_mul(out=to, in0=td, in1=tr)
            nc.gpsimd.dma_start(out=o[:, :], in_=to)
    nc.compile()
    return nc


def _run(key, builder, in_maps, trace=False):
    from concourse.bass_utils import run_bass_kernel_spmd

    if key not in _CACHE:
        _CACHE[key] = builder()
    res = run_bass_kernel_spmd(
        _CACHE[key], in_maps, core_ids=list(range(NCORES)), trace=trace
    )
    return res


def kernel(input, kernel):
    import os

    trace = bool(int(os.environ.get("BASSBLUR_TRACE", "0")))
    inp = np.ascontiguousarray(np.asarray(input, dtype=np.float32))
    ker = np.ascontiguousarray(np.asarray(kernel, dtype=np.float32))
    X16 = ker.reshape(L2, P).astype(np.float16)
    Xf16 = X16.reshape(-1)

    times = []

    # ---- launch 1: inv1 = 1 / column sums of X (PE ones-matmul) -------
    import ml_dtypes

    X8 = ker.reshape(L2, P).astype(ml_dtypes.float8_e4m3fn)
    in1 = [
        {"xp": np.ascontiguousarray(X8[:, m * PS : (m + 1) * PS])}
        for m in range(NCORES)
    ]
    r1 = _run("k1", _build_k1, in1, trace=trace)
    inv1 = np.concatenate([r["inv1"][0] for r in r1.results])
    times.append(r1.exec_time_ns)

    # ---- launch 2: per-chunk stage-2 normalize ------------------------
    # band m covers flat [NB*m, NB*(m+1)); element x there needs
    # inv1[(NB*m + x) % P]; NB % P == PS so the roll shift is PS*m.
    in2 = []
    for m in range(NCORES):
        i1b = np.resize(np.roll(inv1, -(PS * m) % P), NB).reshape(PS, L2)
        in2.append(
            {
                "a2": Xf16[NB * m : NB * (m + 1)].reshape(PS, L2),
                "i1b": np.ascontiguousarray(i1b).astype(np.float16),
            }
        )
    r2 = _run("k2", _build_k2, in2, trace=trace)
    B2 = np.concatenate([r["b2"] for r in r2.results], axis=0)  # (P, 361) f16
    times.append(r2.exec_time_ns)

    # ---- launch 3: final dot over B2T/U flat chunks (PE reduce) -------
    B2Tf = np.ascontiguousarray(B2.T).reshape(-1)
    pad = np.pad(inp[0, 2], L // 2, mode="reflect").astype(np.float16)
    from numpy.lib.stride_tricks import sliding_window_view

    U = np.ascontiguousarray(
        sliding_window_view(pad, (256, 256)).reshape(L2, P)
    )
    Uf = U.reshape(-1)
    in3 = [
        {
            "vT": np.ascontiguousarray(
                B2Tf[NB * m : NB * (m + 1)].reshape(PS, L2).T
            ),
            "uT": np.ascontiguousarray(
                Uf[NB * m : NB * (m + 1)].reshape(PS, L2).T
            ),
        }
        for m in range(NCORES)
    ]
    r3 = _run("k3", _build_k3, in3, trace=trace)
    out = np.concatenate([r["o"].ravel() for r in r3.results])
    times.append(r3.exec_time_ns)

    if trace:
        kernel._last_times_ns = times  # stash for test harness

    return out.reshape(1, 1, 256, 256).astype(np.float32)


def hw_time_estimate_ns():
    """Per-launch HW time from the instruction cost model (TimelineSim).

    NTFF/neuron-profile capture is unavailable under this axon build, so this
    is the principled substitute: the same InstructionCostModel the Tile
    scheduler uses, over the exact BIR that runs on the cores.
    """
    from concourse.timeline_sim import TimelineSim

    out = []
    for key, builder in [("k1", _build_k1), ("k2", _build_k2), ("k3", _build_k3)]:
        if key not in _CACHE:
            _CACHE[key] = builder()
        out.append(int(TimelineSim(_CACHE[key]).simulate()))
    return out
